# revision 5
# baseline (speedup 1.0000x reference)
"""Trainium2 Bass kernel for nn_AttnWithBias_77412490543864.

Math: the reference einsum 'bqnk,bvnd->bqnd' contracts attn over k and v
independently; softmax rows sum to 1, so the attention output reduces exactly
to v.sum(axis=1) broadcast over queries. The whole bias-layer / QK / softmax
computation cancels, leaving:

    vs  = pieces.sum(axis=1) @ Wv + LEN2 * bv            # [B, D]
    s   = LN1(state + vs[:, None, :])                    # [B, LEN1, D]
    h   = relu(s @ W1 + b1)                              # [*, FF]
    out = LN2(s + h @ W2 + b2)                           # [B, LEN1, D]

Sharding: data-parallel over batch, 32 batches per core, 8 cores, no
collectives. FFN matmuls in bf16 (fp32 PSUM accumulation); LayerNorm,
residuals and the Wv projection in fp32.
"""

import numpy as np
import ml_dtypes

B, LEN1, LEN2, D, H, DH, FF, HID = 256, 192, 64, 1024, 8, 128, 4096, 128
EPS = 1e-5
N_CORES = 8
BC = B // N_CORES            # batches per core
R = BC * LEN1                # activation rows per core
PR = BC * LEN2               # pieces rows per core
P = 128
KT = D // P                  # k-tiles over D
MT = FF // P                 # ff tiles
CHUNK = 512
SUB = CHUNK // P             # row sub-tiles per chunk
NCHUNK = R // CHUNK
NT = D // P                  # output d-tiles

_BUILT = None


def _build_nc(reps=1):
    import concourse.mybir as mybir
    import concourse.tile as tile
    from concourse import bacc
    from concourse.masks import make_identity

    f32 = mybir.dt.float32
    bf16 = mybir.dt.bfloat16
    AF = mybir.ActivationFunctionType
    OP = mybir.AluOpType

    nc = bacc.Bacc("TRN2", target_bir_lowering=False, debug=False)

    state_d = nc.dram_tensor("state", [R, D], f32, kind="ExternalInput")
    pieces_d = nc.dram_tensor("pieces", [PR, D], f32, kind="ExternalInput")
    wv_d = nc.dram_tensor("wv", [D, D], f32, kind="ExternalInput")
    bv_d = nc.dram_tensor("bv", [D], f32, kind="ExternalInput")
    w1_d = nc.dram_tensor("w1", [D, FF], bf16, kind="ExternalInput")
    b1_d = nc.dram_tensor("b1", [FF], f32, kind="ExternalInput")
    w2_d = nc.dram_tensor("w2", [FF, D], bf16, kind="ExternalInput")
    b2_d = nc.dram_tensor("b2", [D], bf16, kind="ExternalInput")
    out_d = nc.dram_tensor("out", [R, D], f32, kind="ExternalOutput")

    w1_r = w1_d.ap().rearrange("(k p) f -> p k f", p=P)      # [128, 8, 4096]

    with tile.TileContext(nc) as tc:
        with (
            tc.tile_pool(name="const", bufs=1) as const,
            tc.tile_pool(name="w2pool", bufs=1) as w2pool,
            tc.tile_pool(name="dram", bufs=1, space="DRAM") as dram,
        ):
            # ---- constants / resident weights ----
            ident_bf = const.tile([P, P], bf16)
            make_identity(nc, ident_bf)
            eps_t = const.tile([P, 1], f32)
            nc.vector.memset(eps_t, EPS)
            b1sb = const.tile([P, MT], f32)
            nc.sync.dma_start(out=b1sb, in_=b1_d.ap().rearrange("(m p) -> p m", p=P))
            b2row = const.tile([1, D], bf16)
            nc.sync.dma_start(out=b2row, in_=b2_d.ap().unsqueeze(0))
            ones_row = const.tile([1, P], bf16)
            nc.vector.memset(ones_row, 1.0)

            w2sb = w2pool.tile([P, MT, D], bf16)
            nc.sync.dma_start(out=w2sb, in_=w2_d.ap().rearrange("(m p) d -> p m d", p=P))

            vs_t = dram.tile([BC, D], f32)

            # ---- prologue: vs = pieces.sum(axis=1) @ Wv + 64*bv ----
            with (
                tc.tile_pool(name="pro", bufs=1) as pro,
                tc.tile_pool(name="pro_ps", bufs=2, space="PSUM") as pro_ps,
                tc.tile_pool(name="vs_ps", bufs=2, space="PSUM") as vs_ps,
            ):
                ones_blk = pro.tile([P, 2], f32)
                nc.vector.memset(ones_blk, 0.0)
                nc.vector.memset(ones_blk[0:64, 0:1], 1.0)
                nc.vector.memset(ones_blk[64:128, 1:2], 1.0)
                k64 = pro.tile([1, BC], f32)
                nc.vector.memset(k64, float(LEN2))
                bv_sb = pro.tile([1, D], f32)
                nc.sync.dma_start(out=bv_sb, in_=bv_d.ap().unsqueeze(0))

                pc = pro.tile([P, PR // P, D], f32)
                nc.sync.dma_start(
                    out=pc, in_=pieces_d.ap().rearrange("(t p) d -> p t d", p=P)
                )
                wv_sb = pro.tile([P, KT, D], f32)
                nc.sync.dma_start(
                    out=wv_sb, in_=wv_d.ap().rearrange("(k p) d -> p k d", p=P)
                )

                # psumT[d, b] = sum_t pieces[(b,t), d] via matmul with block-ones
                psumT_sb = pro.tile([P, KT, BC], f32)
                for k in range(KT):
                    pst = pro_ps.tile([P, BC], f32)
                    for t in range(PR // P):
                        nc.tensor.matmul(
                            pst[:, 2 * t:2 * t + 2],
                            lhsT=pc[:, t, k * P:(k + 1) * P],
                            rhs=ones_blk,
                            start=True, stop=True,
                        )
                    nc.scalar.copy(out=psumT_sb[:, k, :], in_=pst)

                vs_sb = pro.tile([BC, D], f32)
                for n in range(2):
                    pvs = vs_ps.tile([BC, 512], f32)
                    for k in range(KT):
                        nc.tensor.matmul(
                            pvs,
                            lhsT=psumT_sb[:, k, :],
                            rhs=wv_sb[:, k, n * 512:(n + 1) * 512],
                            start=(k == 0), stop=False,
                        )
                    nc.tensor.matmul(
                        pvs,
                        lhsT=k64,
                        rhs=bv_sb[:, n * 512:(n + 1) * 512],
                        start=False, stop=True,
                    )
                    nc.scalar.copy(out=vs_sb[:, n * 512:(n + 1) * 512], in_=pvs)
                nc.sync.dma_start(out=vs_t[:, :], in_=vs_sb)

            # ---- main pipeline over row chunks ----
            # Emission order is software-pipelined: stage A (LN1 prep) for
            # chunk c+1 is emitted between chunk c's transposes and matmuls,
            # so DVE prepares the next chunk while PE runs the current one.
            with (
                tc.tile_pool(name="xpool", bufs=3) as xpool,
                tc.tile_pool(name="vbpool", bufs=2) as vbpool,
                tc.tile_pool(name="sbfpool", bufs=2) as sbfpool,
                tc.tile_pool(name="stpool", bufs=1) as stpool,
                tc.tile_pool(name="htpool", bufs=1) as htpool,
                tc.tile_pool(name="w1pool", bufs=4) as w1pool,
                tc.tile_pool(name="statpool", bufs=8) as statpool,
                tc.tile_pool(name="pp1", bufs=3, space="PSUM") as pp1,
                tc.tile_pool(name="pp2", bufs=3, space="PSUM") as pp2,
                tc.tile_pool(name="ppts", bufs=2, space="PSUM") as ppts,
            ):
                def stage_a(c):
                    """x = state + vs ; s = LN1(x) in-place ; s_bf = bf16(s)."""
                    rows = slice(c * CHUNK, (c + 1) * CHUNK)
                    x = xpool.tile([P, SUB, D], f32, tag="x")
                    nc.sync.dma_start(
                        out=x,
                        in_=state_d.ap()[rows, :].rearrange("(i p) d -> p i d", p=P),
                    )
                    s_bf = sbfpool.tile([P, SUB, D], bf16, tag="s_bf")
                    for i in range(SUB):
                        r0 = c * CHUNK + i * P
                        b0 = r0 // LEN1
                        span = min(LEN1 - (r0 % LEN1), P)
                        vsb = vbpool.tile([P, D], f32, tag="vsb")
                        nc.gpsimd.dma_start(
                            out=vsb[0:span, :],
                            in_=vs_t[b0:b0 + 1, :].to_broadcast((span, D)),
                        )
                        if span < P:
                            nc.gpsimd.dma_start(
                                out=vsb[span:P, :],
                                in_=vs_t[b0 + 1:b0 + 2, :].to_broadcast((P - span, D)),
                            )
                        nc.vector.tensor_add(x[:, i, :], x[:, i, :], vsb)

                        stats = statpool.tile([P, 2, 6], f32, tag="stats")
                        mv = statpool.tile([P, 2], f32, tag="mv")
                        for g in range(2):
                            nc.vector.bn_stats(
                                out=stats[:, g, :], in_=x[:, i, g * 512:(g + 1) * 512]
                            )
                        nc.vector.bn_aggr(out=mv, in_=stats)
                        nc.scalar.activation(
                            out=mv[:, 1:2], in_=mv[:, 1:2], func=AF.Sqrt,
                            bias=eps_t, scale=1.0,
                        )
                        nc.vector.reciprocal(out=mv[:, 1:2], in_=mv[:, 1:2])
                        nc.vector.tensor_scalar(
                            out=x[:, i, :], in0=x[:, i, :],
                            scalar1=mv[:, 0:1], scalar2=mv[:, 1:2],
                            op0=OP.subtract, op1=OP.mult,
                        )
                        nc.vector.tensor_copy(out=s_bf[:, i, :], in_=x[:, i, :])
                    return x, s_bf

                for rep in range(reps):
                  xs = {}
                  xs[0] = stage_a(0)
                  for c in range(NCHUNK):
                    x, s_bf = xs.pop(c)

                    # -- transpose s (bf16) -> sT --
                    sT = stpool.tile([P, KT, CHUNK], bf16, tag="sT")
                    for i in range(SUB):
                        for k in range(KT):
                            tpp = ppts.tile([P, P], bf16, tag="tps")
                            nc.tensor.transpose(
                                tpp, s_bf[:, i, k * P:(k + 1) * P], ident_bf
                            )
                            nc.vector.tensor_copy(
                                out=sT[:, k, i * P:(i + 1) * P], in_=tpp
                            )

                    # -- prep next chunk on DVE while PE runs matmuls --
                    if c + 1 < NCHUNK:
                        xs[c + 1] = stage_a(c + 1)

                    # -- mm1: hT = relu(W1^T s^T + b1) --
                    hT = htpool.tile([P, MT, CHUNK], bf16, tag="hT")
                    for m in range(MT):
                        w1t = w1pool.tile([P, KT, P], bf16, tag="w1t")
                        nc.sync.dma_start(
                            out=w1t, in_=w1_r[:, :, m * P:(m + 1) * P]
                        )
                        ph = pp1.tile([P, CHUNK], f32, tag="ph")
                        for k in range(KT):
                            nc.tensor.matmul(
                                ph, lhsT=w1t[:, k, :], rhs=sT[:, k, :],
                                start=(k == 0), stop=(k == KT - 1),
                            )
                        nc.scalar.activation(
                            out=hT[:, m, :], in_=ph, func=AF.Relu,
                            bias=b1sb[:, m:m + 1], scale=1.0,
                        )

                    # -- mm2 (row-major out): y[rows, d] = hT.T @ W2 + b2,
                    #    b2 folded in as an augmented-K ones-row matmul;
                    #    residual s added during PSUM eviction --
                    for i in range(SUB):
                        for dd in range(2):
                            py = pp2.tile([P, 512], f32, tag="py")
                            for m in range(MT):
                                nc.tensor.matmul(
                                    py, lhsT=hT[:, m, i * P:(i + 1) * P],
                                    rhs=w2sb[:, m, dd * 512:(dd + 1) * 512],
                                    start=(m == 0), stop=False,
                                )
                            nc.tensor.matmul(
                                py, lhsT=ones_row,
                                rhs=b2row[:, dd * 512:(dd + 1) * 512],
                                start=False, stop=True,
                            )
                            nc.vector.tensor_add(
                                x[:, i, dd * 512:(dd + 1) * 512],
                                py,
                                x[:, i, dd * 512:(dd + 1) * 512],
                            )
                    for i in range(SUB):
                        stats = statpool.tile([P, 2, 6], f32, tag="stats")
                        mv = statpool.tile([P, 2], f32, tag="mv")
                        for g in range(2):
                            nc.vector.bn_stats(
                                out=stats[:, g, :], in_=x[:, i, g * 512:(g + 1) * 512]
                            )
                        nc.vector.bn_aggr(out=mv, in_=stats)
                        nc.scalar.activation(
                            out=mv[:, 1:2], in_=mv[:, 1:2], func=AF.Sqrt,
                            bias=eps_t, scale=1.0,
                        )
                        nc.vector.reciprocal(out=mv[:, 1:2], in_=mv[:, 1:2])
                        nc.vector.tensor_scalar(
                            out=x[:, i, :], in0=x[:, i, :],
                            scalar1=mv[:, 0:1], scalar2=mv[:, 1:2],
                            op0=OP.subtract, op1=OP.mult,
                        )
                    rows = slice(c * CHUNK, (c + 1) * CHUNK)
                    nc.sync.dma_start(
                        out=out_d.ap()[rows, :].rearrange("(i p) d -> p i d", p=P),
                        in_=x,
                    )

    nc.compile()
    return nc


def get_built(reps=1):
    global _BUILT
    if not isinstance(_BUILT, dict):
        _BUILT = {}
    if reps not in _BUILT:
        _BUILT[reps] = _build_nc(reps)
    return _BUILT[reps]


def _prep_in_maps(state, pieces, params):
    state = np.ascontiguousarray(np.asarray(state, dtype=np.float32))
    pieces = np.ascontiguousarray(np.asarray(pieces, dtype=np.float32))
    p = params
    wv = np.ascontiguousarray(np.asarray(p["v"]["w"], dtype=np.float32))
    bv = np.ascontiguousarray(np.asarray(p["v"]["b"], dtype=np.float32))
    w1 = np.ascontiguousarray(
        np.asarray(p["ff1"]["w"], dtype=np.float32).astype(ml_dtypes.bfloat16)
    )
    b1 = np.ascontiguousarray(np.asarray(p["ff1"]["b"], dtype=np.float32))
    w2 = np.ascontiguousarray(
        np.asarray(p["ff2"]["w"], dtype=np.float32).astype(ml_dtypes.bfloat16)
    )
    b2 = np.ascontiguousarray(
        np.asarray(p["ff2"]["b"], dtype=np.float32).astype(ml_dtypes.bfloat16)
    )

    in_maps = []
    for c in range(N_CORES):
        bs = slice(c * BC, (c + 1) * BC)
        in_maps.append({
            "state": np.ascontiguousarray(state[bs].reshape(R, D)),
            "pieces": np.ascontiguousarray(pieces[bs].reshape(PR, D)),
            "wv": wv, "bv": bv, "w1": w1, "b1": b1, "w2": w2, "b2": b2,
        })
    return in_maps


def _norms_trivial(params):
    for k in ("norm1", "norm2"):
        g = np.asarray(params[k]["g"])
        b = np.asarray(params[k]["b"])
        if not (np.all(g == 1.0) and np.all(b == 0.0)):
            return False
    return True


def _numpy_fallback(state, pieces, params):
    state = np.asarray(state, dtype=np.float32)
    pieces = np.asarray(pieces, dtype=np.float32)
    p = params

    def ln(x, g, b):
        mu = x.mean(-1, keepdims=True)
        var = ((x - mu) ** 2).mean(-1, keepdims=True)
        return (x - mu) / np.sqrt(var + EPS) * np.asarray(g) + np.asarray(b)

    vs = pieces.sum(axis=1) @ np.asarray(p["v"]["w"]) + LEN2 * np.asarray(p["v"]["b"])
    x = state + vs[:, None, :]
    s = ln(x, p["norm1"]["g"], p["norm1"]["b"])
    h = np.maximum(
        s.reshape(-1, D) @ np.asarray(p["ff1"]["w"]) + np.asarray(p["ff1"]["b"]), 0.0
    )
    y = (h @ np.asarray(p["ff2"]["w"]) + np.asarray(p["ff2"]["b"])).reshape(B, LEN1, D)
    return ln(s + y, p["norm2"]["g"], p["norm2"]["b"]).astype(np.float32)


def kernel(state, pieces, params):
    if not _norms_trivial(params) or np.asarray(state).shape != (B, LEN1, D):
        return _numpy_fallback(state, pieces, params)
    try:
        from concourse.bass_utils import run_bass_kernel_spmd

        nc = get_built()
        in_maps = _prep_in_maps(state, pieces, params)
        res = run_bass_kernel_spmd(nc, in_maps, core_ids=list(range(N_CORES)))
        out = np.concatenate(
            [res.results[c]["out"].reshape(BC, LEN1, D) for c in range(N_CORES)],
            axis=0,
        )
        return out
    except Exception as e:  # safety net: never return a wrong-shaped/failed result
        import traceback
        traceback.print_exc()
        print(f"kernel: device path failed ({type(e).__name__}); numpy fallback")
        return _numpy_fallback(state, pieces, params)


# revision 12
# speedup vs baseline: 7.1444x; 7.1444x over previous
"""Trainium2 Bass kernel for nn_AttnWithBias_77412490543864.

Math: the reference einsum 'bqnk,bvnd->bqnd' contracts attn over k and v
independently; softmax rows sum to 1, so the attention output reduces exactly
to v.sum(axis=1) broadcast over queries. The whole bias-layer / QK / softmax
computation cancels, leaving:

    vs  = pieces.sum(axis=1) @ Wv + LEN2 * bv            # [B, D]
    s   = LN1(state + vs[:, None, :])                    # [B, LEN1, D]
    h   = relu(s @ W1 + b1)                              # [*, FF]
    out = LN2(s + h @ W2 + b2)                           # [B, LEN1, D]

Sharding: data-parallel over batch, 32 batches per core, 8 cores, no
collectives. FFN matmuls in bf16 (fp32 PSUM accumulation); LayerNorm,
residuals and the Wv projection in fp32.
"""

import numpy as np
import ml_dtypes

B, LEN1, LEN2, D, H, DH, FF, HID = 256, 192, 64, 1024, 8, 128, 4096, 128
EPS = 1e-5
N_CORES = 8
BC = B // N_CORES            # batches per core
R = BC * LEN1                # activation rows per core
PR = BC * LEN2               # pieces rows per core
P = 128
KT = D // P                  # k-tiles over D
MT = FF // P                 # ff tiles
CHUNK = 512
SUB = CHUNK // P             # row sub-tiles per chunk
NCHUNK = R // CHUNK
NT = D // P                  # output d-tiles

_BUILT = None


def _build_nc(reps=1):
    import concourse.mybir as mybir
    import concourse.tile as tile
    from concourse import bacc

    f32 = mybir.dt.float32
    bf16 = mybir.dt.bfloat16
    AF = mybir.ActivationFunctionType
    OP = mybir.AluOpType

    nc = bacc.Bacc("TRN2", target_bir_lowering=False, debug=False)

    state_d = nc.dram_tensor("state", [R, D], f32, kind="ExternalInput")
    pieces_d = nc.dram_tensor("pieces", [PR, D], f32, kind="ExternalInput")
    wv_d = nc.dram_tensor("wv", [D, D], f32, kind="ExternalInput")
    bv_d = nc.dram_tensor("bv", [D], f32, kind="ExternalInput")
    w1_d = nc.dram_tensor("w1", [D, FF], bf16, kind="ExternalInput")
    b1_d = nc.dram_tensor("b1", [FF], f32, kind="ExternalInput")
    w2_d = nc.dram_tensor("w2", [FF, D], bf16, kind="ExternalInput")
    b2_d = nc.dram_tensor("b2", [D], bf16, kind="ExternalInput")
    out_d = nc.dram_tensor("out", [R, D], f32, kind="ExternalOutput")

    w1_r = w1_d.ap().rearrange("(k p) f -> p k f", p=P)      # [128, 8, 4096]

    with tile.TileContext(nc) as tc:
        with (
            tc.tile_pool(name="const", bufs=1) as const,
            tc.tile_pool(name="w2pool", bufs=1) as w2pool,
            tc.tile_pool(name="dram", bufs=1, space="DRAM") as dram,
        ):
            # ---- constants / resident weights ----
            eps_t = const.tile([P, 1], f32)
            nc.vector.memset(eps_t, EPS)
            b1sb = const.tile([P, MT], f32)
            nc.sync.dma_start(out=b1sb, in_=b1_d.ap().rearrange("(m p) -> p m", p=P))
            b2row = const.tile([1, D], bf16)
            nc.sync.dma_start(out=b2row, in_=b2_d.ap().unsqueeze(0))
            ones_row = const.tile([1, P], bf16)
            nc.vector.memset(ones_row, 1.0)


            vs_t = dram.tile([BC, D], f32)

            # ---- prologue: vs = pieces.sum(axis=1) @ Wv + 64*bv ----
            with (
                tc.tile_pool(name="pro", bufs=1) as pro,
                tc.tile_pool(name="pro_ps", bufs=2, space="PSUM") as pro_ps,
                tc.tile_pool(name="vs_ps", bufs=2, space="PSUM") as vs_ps,
            ):
                pc = pro.tile([P, PR // P, D], f32)
                nc.sync.dma_start(
                    out=pc, in_=pieces_d.ap().rearrange("(t p) d -> p t d", p=P)
                )
                wv_sb = pro.tile([P, KT, D], f32)
                nc.sync.dma_start(
                    out=wv_sb, in_=wv_d.ap().rearrange("(k p) d -> p k d", p=P)
                )
                ones_blk = pro.tile([P, 2], f32)
                nc.vector.memset(ones_blk, 0.0)
                nc.vector.memset(ones_blk[0:64, 0:1], 1.0)
                nc.vector.memset(ones_blk[64:128, 1:2], 1.0)
                k64 = pro.tile([1, BC], f32)
                nc.vector.memset(k64, float(LEN2))
                bv_sb = pro.tile([1, D], f32)
                nc.sync.dma_start(out=bv_sb, in_=bv_d.ap().unsqueeze(0))

                # psumT[d, b] = sum_t pieces[(b,t), d] via matmul with block-ones
                psumT_sb = pro.tile([P, KT, BC], f32)
                for k in range(KT):
                    pst = pro_ps.tile([P, BC], f32)
                    for t in range(PR // P):
                        nc.tensor.matmul(
                            pst[:, 2 * t:2 * t + 2],
                            lhsT=pc[:, t, k * P:(k + 1) * P],
                            rhs=ones_blk,
                            start=True, stop=True,
                        )
                    nc.scalar.copy(out=psumT_sb[:, k, :], in_=pst)

                vs_sb = pro.tile([BC, D], f32)
                for n in range(2):
                    pvs = vs_ps.tile([BC, 512], f32)
                    for k in range(KT):
                        nc.tensor.matmul(
                            pvs,
                            lhsT=psumT_sb[:, k, :],
                            rhs=wv_sb[:, k, n * 512:(n + 1) * 512],
                            start=(k == 0), stop=False,
                        )
                    nc.tensor.matmul(
                        pvs,
                        lhsT=k64,
                        rhs=bv_sb[:, n * 512:(n + 1) * 512],
                        start=False, stop=True,
                    )
                    nc.scalar.copy(out=vs_sb[:, n * 512:(n + 1) * 512], in_=pvs)
                nc.sync.dma_start(out=vs_t[:, :], in_=vs_sb)

            w2sb = w2pool.tile([P, MT, D], bf16)
            nc.sync.dma_start(out=w2sb, in_=w2_d.ap().rearrange("(m p) d -> p m d", p=P))

            # ---- main pipeline over row chunks ----
            # Emission order is software-pipelined: stage A (LN1 prep) for
            # chunk c+1 is emitted between chunk c's transposes and matmuls,
            # so DVE prepares the next chunk while PE runs the current one.
            with (
                tc.tile_pool(name="xpool", bufs=3) as xpool,
                tc.tile_pool(name="vbpool", bufs=2) as vbpool,
                tc.tile_pool(name="sbfpool", bufs=2) as sbfpool,
                tc.tile_pool(name="stpool", bufs=1) as stpool,
                tc.tile_pool(name="htpool", bufs=1) as htpool,
                tc.tile_pool(name="w1pool", bufs=6) as w1pool,
                tc.tile_pool(name="statpool", bufs=8) as statpool,
                tc.tile_pool(name="pp1", bufs=3, space="PSUM") as pp1,
                tc.tile_pool(name="pp2", bufs=3, space="PSUM") as pp2,
            ):
                def stage_a(c):
                    """x = state + vs ; s = LN1(x) in-place ; s_bf = bf16(s)."""
                    rows = slice(c * CHUNK, (c + 1) * CHUNK)
                    x = xpool.tile([P, SUB, D], f32, tag="x")
                    nc.scalar.dma_start(
                        out=x,
                        in_=state_d.ap()[rows, :].rearrange("(i p) d -> p i d", p=P),
                    )
                    s_bf = sbfpool.tile([P, SUB, D], bf16, tag="s_bf")
                    for i in range(SUB):
                        r0 = c * CHUNK + i * P
                        b0 = r0 // LEN1
                        span = min(LEN1 - (r0 % LEN1), P)
                        vsb = vbpool.tile([P, D], f32, tag="vsb")
                        nc.gpsimd.dma_start(
                            out=vsb[0:span, :],
                            in_=vs_t[b0:b0 + 1, :].to_broadcast((span, D)),
                        )
                        if span < P:
                            nc.gpsimd.dma_start(
                                out=vsb[span:P, :],
                                in_=vs_t[b0 + 1:b0 + 2, :].to_broadcast((P - span, D)),
                            )
                        nc.vector.tensor_add(x[:, i, :], x[:, i, :], vsb)

                        stats = statpool.tile([P, 2, 6], f32, tag="stats")
                        mv = statpool.tile([P, 2], f32, tag="mv")
                        for g in range(2):
                            nc.vector.bn_stats(
                                out=stats[:, g, :], in_=x[:, i, g * 512:(g + 1) * 512]
                            )
                        nc.vector.bn_aggr(out=mv, in_=stats)
                        nc.scalar.activation(
                            out=mv[:, 1:2], in_=mv[:, 1:2], func=AF.Sqrt,
                            bias=eps_t, scale=1.0,
                        )
                        nc.vector.reciprocal(out=mv[:, 1:2], in_=mv[:, 1:2])
                        nc.vector.tensor_scalar(
                            out=x[:, i, :], in0=x[:, i, :],
                            scalar1=mv[:, 0:1], scalar2=mv[:, 1:2],
                            op0=OP.subtract, op1=OP.mult,
                        )
                        nc.vector.tensor_copy(out=s_bf[:, i, :], in_=x[:, i, :])
                    return x, s_bf

                for rep in range(reps):
                  xs = {}
                  xs[0] = stage_a(0)
                  for c in range(NCHUNK):
                    x, s_bf = xs.pop(c)

                    # -- transpose s (bf16) -> sT via X-bar DMA transpose --
                    sT = stpool.tile([P, KT, CHUNK], bf16, tag="sT")
                    for i in range(SUB):
                        nc.sync.dma_start_transpose(
                            out=sT[:, :, i * P:(i + 1) * P], in_=s_bf[:, i, :]
                        )

                    # -- prep next chunk on DVE while PE runs matmuls --
                    if c + 1 < NCHUNK:
                        xs[c + 1] = stage_a(c + 1)

                    # -- mm1: hT = relu(W1^T s^T + b1) --
                    hT = htpool.tile([P, MT, CHUNK], bf16, tag="hT")
                    w1ts = []
                    for m in range(MT):
                        w1t = w1pool.tile([P, KT, P], bf16, tag="w1t")
                        nc.sync.dma_start(
                            out=w1t, in_=w1_r[:, :, m * P:(m + 1) * P]
                        )
                        w1ts.append(w1t)
                    for m in range(MT):
                        w1t = w1ts[m]
                        ph = pp1.tile([P, CHUNK], f32, tag="ph")
                        for k in range(KT):
                            nc.tensor.matmul(
                                ph, lhsT=w1t[:, k, :], rhs=sT[:, k, :],
                                start=(k == 0), stop=(k == KT - 1),
                            )
                        nc.scalar.activation(
                            out=hT[:, m, :], in_=ph, func=AF.Relu,
                            bias=b1sb[:, m:m + 1], scale=1.0,
                        )

                    # -- mm2 (row-major out): y[rows, d] = hT.T @ W2 + b2,
                    #    b2 folded in as an augmented-K ones-row matmul;
                    #    residual s added during PSUM eviction --
                    for i in range(SUB):
                        for dd in range(2):
                            py = pp2.tile([P, 512], f32, tag="py")
                            for m in range(MT):
                                nc.tensor.matmul(
                                    py, lhsT=hT[:, m, i * P:(i + 1) * P],
                                    rhs=w2sb[:, m, dd * 512:(dd + 1) * 512],
                                    start=(m == 0), stop=False,
                                )
                            nc.tensor.matmul(
                                py, lhsT=ones_row,
                                rhs=b2row[:, dd * 512:(dd + 1) * 512],
                                start=False, stop=True,
                            )
                            nc.vector.tensor_add(
                                x[:, i, dd * 512:(dd + 1) * 512],
                                py,
                                x[:, i, dd * 512:(dd + 1) * 512],
                            )
                    for i in range(SUB):
                        stats = statpool.tile([P, 2, 6], f32, tag="stats")
                        mv = statpool.tile([P, 2], f32, tag="mv")
                        for g in range(2):
                            nc.vector.bn_stats(
                                out=stats[:, g, :], in_=x[:, i, g * 512:(g + 1) * 512]
                            )
                        nc.vector.bn_aggr(out=mv, in_=stats)
                        nc.scalar.activation(
                            out=mv[:, 1:2], in_=mv[:, 1:2], func=AF.Sqrt,
                            bias=eps_t, scale=1.0,
                        )
                        nc.vector.reciprocal(out=mv[:, 1:2], in_=mv[:, 1:2])
                        nc.vector.tensor_scalar(
                            out=x[:, i, :], in0=x[:, i, :],
                            scalar1=mv[:, 0:1], scalar2=mv[:, 1:2],
                            op0=OP.subtract, op1=OP.mult,
                        )
                    rows = slice(c * CHUNK, (c + 1) * CHUNK)
                    nc.scalar.dma_start(
                        out=out_d.ap()[rows, :].rearrange("(i p) d -> p i d", p=P),
                        in_=x,
                    )

    nc.compile()
    return nc


def get_built(reps=1):
    global _BUILT
    if not isinstance(_BUILT, dict):
        _BUILT = {}
    if reps not in _BUILT:
        _BUILT[reps] = _build_nc(reps)
    return _BUILT[reps]


def _prep_in_maps(state, pieces, params):
    state = np.ascontiguousarray(np.asarray(state, dtype=np.float32))
    pieces = np.ascontiguousarray(np.asarray(pieces, dtype=np.float32))
    p = params
    wv = np.ascontiguousarray(np.asarray(p["v"]["w"], dtype=np.float32))
    bv = np.ascontiguousarray(np.asarray(p["v"]["b"], dtype=np.float32))
    w1 = np.ascontiguousarray(
        np.asarray(p["ff1"]["w"], dtype=np.float32).astype(ml_dtypes.bfloat16)
    )
    b1 = np.ascontiguousarray(np.asarray(p["ff1"]["b"], dtype=np.float32))
    w2 = np.ascontiguousarray(
        np.asarray(p["ff2"]["w"], dtype=np.float32).astype(ml_dtypes.bfloat16)
    )
    b2 = np.ascontiguousarray(
        np.asarray(p["ff2"]["b"], dtype=np.float32).astype(ml_dtypes.bfloat16)
    )

    in_maps = []
    for c in range(N_CORES):
        bs = slice(c * BC, (c + 1) * BC)
        in_maps.append({
            "state": np.ascontiguousarray(state[bs].reshape(R, D)),
            "pieces": np.ascontiguousarray(pieces[bs].reshape(PR, D)),
            "wv": wv, "bv": bv, "w1": w1, "b1": b1, "w2": w2, "b2": b2,
        })
    return in_maps


def _norms_trivial(params):
    for k in ("norm1", "norm2"):
        g = np.asarray(params[k]["g"])
        b = np.asarray(params[k]["b"])
        if not (np.all(g == 1.0) and np.all(b == 0.0)):
            return False
    return True


def _numpy_fallback(state, pieces, params):
    state = np.asarray(state, dtype=np.float32)
    pieces = np.asarray(pieces, dtype=np.float32)
    p = params

    def ln(x, g, b):
        mu = x.mean(-1, keepdims=True)
        var = ((x - mu) ** 2).mean(-1, keepdims=True)
        return (x - mu) / np.sqrt(var + EPS) * np.asarray(g) + np.asarray(b)

    vs = pieces.sum(axis=1) @ np.asarray(p["v"]["w"]) + LEN2 * np.asarray(p["v"]["b"])
    x = state + vs[:, None, :]
    s = ln(x, p["norm1"]["g"], p["norm1"]["b"])
    h = np.maximum(
        s.reshape(-1, D) @ np.asarray(p["ff1"]["w"]) + np.asarray(p["ff1"]["b"]), 0.0
    )
    y = (h @ np.asarray(p["ff2"]["w"]) + np.asarray(p["ff2"]["b"])).reshape(B, LEN1, D)
    return ln(s + y, p["norm2"]["g"], p["norm2"]["b"]).astype(np.float32)


def kernel(state, pieces, params):
    if not _norms_trivial(params) or np.asarray(state).shape != (B, LEN1, D):
        return _numpy_fallback(state, pieces, params)
    try:
        from concourse.bass_utils import run_bass_kernel_spmd

        nc = get_built()
        in_maps = _prep_in_maps(state, pieces, params)
        res = run_bass_kernel_spmd(nc, in_maps, core_ids=list(range(N_CORES)))
        out = np.concatenate(
            [res.results[c]["out"].reshape(BC, LEN1, D) for c in range(N_CORES)],
            axis=0,
        )
        return out
    except Exception as e:  # safety net: never return a wrong-shaped/failed result
        import traceback
        traceback.print_exc()
        print(f"kernel: device path failed ({type(e).__name__}); numpy fallback")
        return _numpy_fallback(state, pieces, params)
